# revision 74
# baseline (speedup 1.0000x reference)
"""Causal self-attention with RoPE on 8 trn2 NeuronCores.

Sharding: core = (batch, head-half). Each of the 8 cores handles one batch
(b = core//2) and 6 of the 12 heads (hh = core%2). Each core computes a
partial output projection (its heads' contribution to y @ Wproj, fp16); the
host sums the two partials per batch in fp32.

Device kernel (identical SPMD program on every core), emission interleaved so
the exp stream (the second-busiest engine) never starves:
  - v = x @ Wv in natural s-major layout (fp16), with an appended ones column
    per head whose matmul row yields the softmax denominator for free.
  - qT/kT = (x @ Wq/Wk)^T in d-major fp16 with RoPE applied via
    stream_shuffle: the head dim is pre-permuted host-side so rotate-half
    partners sit in adjacent even/odd lanes (scores are invariant to that
    permutation). The sin-path multiply/add run on the otherwise-idle GPSIMD
    engine (first chunk: all-DVE fast path for startup latency). qk chunks
    are prefetched ~2 attention chunks ahead so RoPE latency never gates the
    exp stream.
  - flash-style causal attention per head-pair: S^T blocks (k-partition x
    q-free) via row-packed K=64 matmuls, one 3D-AP exp per (pair, chunk, kb)
    covering both heads, diagonal mask multiply as one fp16 DVE op.
  - AV with e as the *stationary* operand: y_q[128q x 65] accumulates [v|1]
    (65 moving cols) over k-blocks kb<=j per 128-wide q-subtile j -- half the
    PE rows of the v-stationary orientation since stationary loads are free.
    Col 64 is the softmax denominator: reciprocal + tensor_scalar_mul
    normalize, then an XBAR dma transpose (idle DMA engines) produces the
    d-major yTn tile for the output projection.
  - out_partial = yTn^T @ Wp (fp16 weights/activations, fp16 output DMA),
    interleaved with attention so the PE never drains; output tiles DMA out
    as they finish.
Inputs land via one batched DMA per logical tensor, split across both HWDGE
queues and ordered so the v-path and pair-0 q/k deps arrive first.
"""
import contextlib

import numpy as np

import concourse.bacc as bacc
import concourse.mybir as mybir
import concourse.tile as tile
from concourse import bass_utils

F32 = mybir.dt.float32
F16 = mybir.dt.float16

B, S, C, H, D = 4, 2048, 768, 12, 64
HPC = H // 2          # heads per core = 6
HP = HPC // 2         # head pairs per core = 3
KC = C // 128         # contraction tiles over C = 6
NST = S // 128        # 128-row s tiles = 16
NSC = S // 512        # 512-wide s chunks = 4
ROPE_BASE = 10000.0

EVEN_ODD_MASK = [x for j in range(16) for x in (2 * j + 1, 2 * j)]


def build_program():
    nc = bacc.Bacc("TRN2", target_bir_lowering=False, debug=False)
    xT_d = nc.dram_tensor("xT", [C, S], F16, kind="ExternalInput").ap()
    wqk_d = nc.dram_tensor("wqk", [C, 768], F16, kind="ExternalInput").ap()
    wv_d = nc.dram_tensor("wv", [C, 384], F16, kind="ExternalInput").ap()
    wp_d = nc.dram_tensor("wp", [384, C], F16, kind="ExternalInput").ap()
    cos_d = nc.dram_tensor("cosT", [128, S], F16, kind="ExternalInput").ap()
    sin_d = nc.dram_tensor("sinA", [128, S], F16, kind="ExternalInput").ap()
    mask_d = nc.dram_tensor("mask01", [128, 256], F16, kind="ExternalInput").ap()
    eye_d = nc.dram_tensor("eye", [128, 128], F32, kind="ExternalInput").ap()
    out_d = nc.dram_tensor("out", [S, C], F16, kind="ExternalOutput").ap()

    with tile.TileContext(nc) as tc, contextlib.ExitStack() as top:
        sb = top.enter_context(tc.tile_pool(name="sb", bufs=1))
        ps = top.enter_context(tc.tile_pool(name="ps", bufs=1, space="PSUM"))

        qkT = [sb.tile([128, S], F16, name=f"qkT{i}", tag=f"qkT{i}") for i in range(6)]
        vones = [
            sb.tile([128, HPC * 65], F16, name=f"vones{i}", tag=f"vones{i}")
            for i in range(NST)
        ]
        yTn = [sb.tile([128, S], F16, name=f"yTn{i}", tag=f"yTn{i}") for i in range(HP)]
        mask2 = sb.tile([128, 256], F16, name="mask2", tag="mask2")
        # xTc[sc][:, 512*kc : 512*(kc+1)] holds x^T rows kc*128..  cols of
        # s-chunk sc; one batched DMA per chunk.
        xTc = [
            sb.tile([128, KC * 512], F16, name=f"xTc{i}", tag=f"xTc{i}")
            for i in range(NSC)
        ]
        wqkt = sb.tile([128, KC * 768], F16, name="wqkt", tag="wqkt")
        wvt = sb.tile([128, KC * 384], F16, name="wvt", tag="wvt")
        wpt = sb.tile([128, HP * 768], F16, name="wpt", tag="wpt")
        cosT = sb.tile([128, S], F16, name="cosT", tag="cosT")
        sinA = sb.tile([128, S], F16, name="sinA", tag="sinA")
        ones6 = sb.tile([128, HPC], F16, name="ones6", tag="ones6")
        eyef = sb.tile([128, 128], F32, name="eyef", tag="eyef")

        def xT(kc, sl):
            """moving-operand view of x^T k-tile kc over s-slice sl"""
            sc = sl.start // 512
            off = sl.start - 512 * sc
            return xTc[sc][:, 512 * kc + off : 512 * kc + off + (sl.stop - sl.start)]

        # batched loads, split across both HWDGE queues so the v-path
        # (wv + x chunk 0) is resident ~2us in.
        src_x = xT_d.rearrange("(k p) s -> p k s", p=128)
        src_wqk = wqk_d.rearrange("(k p) n -> p k n", p=128)
        src_wv = wv_d.rearrange("(k p) n -> p k n", p=128)
        src_wp = wp_d.rearrange("(k p) n -> p k n", p=128)
        # v_tile(0) needs only cols 0:128 of each k-tile: land those first
        x0v = xTc[0][:].rearrange("p (k s) -> p k s", s=512)
        nc.sync.dma_start(x0v[:, :, 0:128], src_x[:, :, 0:128])
        nc.scalar.dma_start(
            wvt[:].rearrange("p (k n) -> p k n", n=384), src_wv
        )
        nc.sync.dma_start(x0v[:, :, 128:512], src_x[:, :, 128:512])
        # wqk in two waves: m-tiles {0,3} (pair 0's q and k) first so
        # attn(0,0) unblocks before the rest of wqk lands.
        wqk3 = wqkt[:].rearrange("p (k n) -> p k n", n=768)
        nc.sync.dma_start(wqk3[:, :, 0:128], src_wqk[:, :, 0:128])
        nc.sync.dma_start(wqk3[:, :, 384:512], src_wqk[:, :, 384:512])
        nc.scalar.dma_start(cosT[:, 0:512], cos_d[:, 0:512])
        nc.scalar.dma_start(sinA[:, 0:512], sin_d[:, 0:512])
        nc.scalar.dma_start(mask2[:], mask_d[:])
        nc.scalar.dma_start(cosT[:, 512:2048], cos_d[:, 512:2048])
        nc.scalar.dma_start(sinA[:, 512:2048], sin_d[:, 512:2048])
        nc.sync.dma_start(wqk3[:, :, 128:384], src_wqk[:, :, 128:384])
        nc.sync.dma_start(wqk3[:, :, 512:768], src_wqk[:, :, 512:768])
        for sc in range(1, NSC):
            nc.sync.dma_start(
                xTc[sc][:].rearrange("p (k s) -> p k s", s=512),
                src_x[:, :, 512 * sc : 512 * (sc + 1)],
            )
        nc.scalar.dma_start(
            wpt[:].rearrange("p (k n) -> p k n", n=768), src_wp
        )
        nc.scalar.dma_start(eyef[:], eye_d[:])
        nc.gpsimd.memset(ones6[:], 1.0)

        def qk_sc(m, sc, fast=False):
            """(x @ Wq/Wk)^T m-tile, q-chunk sc, with RoPE, into qkT[m].

            cos-path mul + shuffle on DVE (PSUM reads); sin-path mul and the
            final add on the idle GPSIMD engine (all-SBUF operands). fast=True
            (startup) runs everything on DVE at high priority to minimize
            latency to the first attention chunk.
            """
            sl = slice(512 * sc, 512 * (sc + 1))
            ctx = tc.high_priority(offset=150) if fast else contextlib.nullcontext()
            with ctx:
                qkps_t = ps.tile([128, 512], F32, name="qkps", tag="pq", bufs=2)
                qkps = qkps_t[:, 0:512]
                for kc in range(KC):
                    nc.tensor.matmul(
                        qkps,
                        wqkt[:, 768 * kc + 128 * m : 768 * kc + 128 * (m + 1)],
                        xT(kc, sl),
                        start=(kc == 0),
                        stop=(kc == KC - 1),
                    )
                qkc = sb.tile([128, 512], F16, name="qkc", tag="qkc", bufs=3)
                shufT = sb.tile([128, 512], F32, name="shufT", tag="shufT", bufs=3)
                shufm = sb.tile([128, 512], F16, name="shufm", tag="shufm", bufs=3)
                nc.vector.tensor_mul(qkc[:], qkps, cosT[:, sl])
                nc.vector.stream_shuffle(shufT[:], qkps, EVEN_ODD_MASK)
                if fast:
                    with nc.allow_low_precision(reason="fp16 rope"):
                        nc.vector.tensor_mul(shufm[:], shufT[:], sinA[:, sl])
                        nc.vector.tensor_add(qkT[m][:, sl], qkc[:], shufm[:])
                else:
                    nc.gpsimd.tensor_mul(shufm[:], shufT[:], sinA[:, sl])
                    nc.gpsimd.tensor_add(qkT[m][:, sl], qkc[:], shufm[:])

        def v_tile(st):
            """v s-tile (fp16, with ones columns) into vones[st].

            Early tiles evacuate via the Act engine (idle during startup);
            later ones via DVE (Act is exp-saturated by then).
            """
            vps_t = ps.tile([128, 512], F32, name="vps", tag="pq", bufs=2)
            vps = vps_t[:, 0:384]
            for kc in range(KC):
                nc.tensor.matmul(
                    vps,
                    xT(kc, slice(128 * st, 128 * (st + 1))),
                    wvt[:, 384 * kc : 384 * (kc + 1)],
                    start=(kc == 0),
                    stop=(kc == KC - 1),
                )
            v3 = vones[st][:].rearrange("p (h w) -> p h w", w=65)
            nc.vector.tensor_copy(v3[:, :, 0:64], vps.rearrange("p (h w) -> p h w", w=64))
            nc.vector.tensor_copy(v3[:, :, 64:65], ones6[:].unsqueeze(2))

        def attn_steps(hp, c, slot, last=False):
            """Generator: causal attention for q-chunk c of head pair hp,
            yielding after each k-block so two chunks (distinct sT slots)
            can interleave -- one chunk's scores run on the PE while the
            other's exp runs on Act.

            Scores/exp per k-block; AV runs e-stationary per q-subtile j
            (global 128-tile index): y_q[128q x 65] accumulates [v|1] (65
            moving cols) over k-blocks kb<=j; col 64 is the denominator;
            reciprocal+scalar-mul normalize, then an XBAR dma transpose
            writes the d-major yTn tile.
            """
            qTt, kTt = qkT[hp], qkT[HP + hp]
            eTs = []
            for kb in range(4 * c + 4):
                off = max(0, 128 * kb - 512 * c)
                qsl = slice(512 * c + off, 512 * (c + 1))
                ksl = slice(128 * kb, 128 * (kb + 1))
                with tc.high_priority(offset=150):
                    sT = ps.tile([128, 1024], F32, name="sT", tag="sT", bufs=2)
                    nc.tensor.matmul(
                        sT[:, off:512], kTt[0:64, ksl], qTt[0:64, qsl],
                        start=True, stop=True, tile_position=(0, 0),
                    )
                    nc.tensor.matmul(
                        sT[:, 512 + off : 1024], kTt[64:128, ksl], qTt[64:128, qsl],
                        start=True, stop=True, tile_position=(64, 0),
                    )
                    eT = sb.tile([128, 1024], F16, name="eT", tag=f"eT{kb}", bufs=4 if kb < 4 else (3 if kb < 8 else 2))
                    eTs.append(eT)
                    in3 = sT[:].rearrange("p (b w) -> p b w", b=2)[:, :, off:512]
                    out3 = eT[:].rearrange("p (b w) -> p b w", b=2)[:, :, off:512]
                    nc.scalar.activation(
                        out3, in3, mybir.ActivationFunctionType.Exp, scale=D**-0.5
                    )
                    if kb >= 4 * c:  # diagonal block: causal mask multiply (both
                        # heads in one fp16 DVE op via the 3D view)
                        e3 = eT[:].rearrange("p (b w) -> p b w", b=2)[:, :, off : off + 128]
                        m3 = mask2[:].rearrange("p (b w) -> p b w", b=2)
                        nc.vector.tensor_mul(e3, e3, m3)
                if kb >= 4 * c:
                    # q-subtile j = kb is fully exp'd: AV + evac for it
                    j = kb
                    jj = j - 4 * c
                    pe_tp = last and kb == 4 * c + 3  # final qsub of the whole
                    # kernel: PE-transpose via the freed sT banks (saves the
                    # ~2.4us XBAR dma-transpose latency on the critical tail)
                    if pe_tp:
                        y16 = sb.tile([128, 128], F32, name="y32", tag="y32", bufs=1)
                    else:
                        y16 = sb.tile([128, 128], F16, name="y16", tag="y16", bufs=6)
                    for h in range(2):
                        yq_t = ps.tile(
                            [128, 65], F32, name="yq", tag=f"yq{h}", bufs=1
                        )
                        yq = yq_t[:, 0:65]
                        with tc.high_priority(offset=150):
                            for kb2 in range(j + 1):
                                nc.tensor.matmul(
                                    yq,
                                    eTs[kb2][:, 512 * h + 128 * jj : 512 * h + 128 * jj + 128],
                                    vones[kb2][:, 65 * (2 * hp + h) : 65 * (2 * hp + h) + 65],
                                    start=(kb2 == 0),
                                    stop=(kb2 == j),
                                )
                        rec = sb.tile([128, 1], F32, name="rec", tag=f"rec{h}", bufs=4)
                        with tc.high_priority(offset=150):
                            nc.vector.reciprocal(rec[:], yq[:, 64:65])
                            with nc.allow_low_precision(reason="fp16 attention output"):
                                nc.vector.tensor_scalar_mul(
                                    y16[:, 64 * h : 64 * (h + 1)],
                                    yq[:, 0:64],
                                    rec[:, 0:1],
                                )
                    if pe_tp:
                        tp_t = ps.tile([128, 1024], F32, name="tp", tag="sT", bufs=2)
                        nc.tensor.transpose(tp_t[:, 0:128], y16[:], eyef[:])
                        with nc.allow_low_precision(reason="fp16 yTn"):
                            nc.vector.tensor_copy(
                                yTn[hp][:, 128 * j : 128 * (j + 1)], tp_t[:, 0:128]
                            )
                    else:
                        nc.sync.dma_start_transpose(
                            yTn[hp][:, 128 * j : 128 * (j + 1)], y16[:]
                        )
                yield

        def attn_pair(a, b=None, last=False):
            """Interleave two chunks' k-block steps on alternating sT slots."""
            gens = [attn_steps(a[0], a[1], 0, last=last)]
            if b is not None:
                gens.append(attn_steps(b[0], b[1], 1))
            while gens:
                done = []
                for g in gens:
                    if next(g, "END") == "END":
                        done.append(g)
                for g in done:
                    gens.remove(g)

        def attn_hi(hp, c, last=False):
            attn_pair((hp, c), last=last)

        def proj_tile(st, tail=False):
            """Output projection s-tile. tail=True routes the out-DMA via the
            Act HWDGE queue (idle at the end) so its semaphore wait cannot
            block later dma transposes on the SP queue."""
            osb = sb.tile([128, 768], F16, name="osb", tag="osb", bufs=3)
            for half in range(2):
                ops_t = ps.tile([128, 512], F32, name="ops", tag="pq", bufs=2)
                ops_ = ops_t[:, 0:384]
                for t in range(HP):
                    nc.tensor.matmul(
                        ops_,
                        yTn[t][:, 128 * st : 128 * (st + 1)],
                        wpt[:, 768 * t + 384 * half : 768 * t + 384 * (half + 1)],
                        start=(t == 0),
                        stop=(t == HP - 1),
                    )
                with nc.allow_low_precision(reason="fp16 partial output"):
                    nc.vector.tensor_copy(osb[:, 384 * half : 384 * (half + 1)], ops_)
            q = nc.scalar if tail else nc.sync
            q.dma_start(out_d[128 * st : 128 * (st + 1), :], osb[:])

        for st in range(0, 4):
            v_tile(st)
        qk_sc(0, 0, fast=True)
        qk_sc(3, 0, fast=True)
        qk_sc(1, 0)
        qk_sc(4, 0)
        qk_sc(2, 0)
        qk_sc(5, 0)
        attn_hi(0, 0)
        qk_sc(0, 1)
        qk_sc(3, 1)
        qk_sc(1, 1)
        qk_sc(4, 1)
        qk_sc(2, 1)
        qk_sc(5, 1)
        attn_hi(1, 0)
        qk_sc(0, 2)
        qk_sc(3, 2)
        attn_hi(2, 0)
        for st in range(4, 8):
            v_tile(st)
        qk_sc(1, 2)
        qk_sc(4, 2)
        attn_hi(0, 1)
        qk_sc(2, 2)
        qk_sc(5, 2)
        attn_hi(1, 1)
        qk_sc(0, 3)
        qk_sc(3, 3)
        attn_hi(2, 1)
        for st in range(8, 12):
            v_tile(st)
        for st in range(0, 4):
            proj_tile(st)
        qk_sc(1, 3)
        qk_sc(4, 3)
        attn_hi(0, 2)
        qk_sc(2, 3)
        qk_sc(5, 3)
        attn_hi(1, 2)
        attn_hi(2, 2)
        for st in range(12, 16):
            v_tile(st)
        for st in range(4, 8):
            proj_tile(st)
        attn_hi(0, 3)
        attn_hi(1, 3)
        attn_hi(2, 3, last=True)
        for st in range(8, 12):
            proj_tile(st)
        for st in range(12, 16):
            proj_tile(st)

    nc.compile()
    return nc


def _rope_tables():
    """cosT/sinA in the even/odd-interleaved d order, tiled to 128 partitions."""
    j = np.arange(32, dtype=np.float64)
    theta = ROPE_BASE ** (-2.0 * j / D)
    pos = np.arange(S, dtype=np.float64)
    freqs = np.outer(theta, pos)  # (32, S)
    cos = np.cos(freqs)
    sin = np.sin(freqs)
    cosT = np.empty((64, S), np.float32)
    sinA = np.empty((64, S), np.float32)
    cosT[0::2] = cos
    cosT[1::2] = cos
    sinA[0::2] = -sin
    sinA[1::2] = sin
    return np.tile(cosT, (2, 1)).copy(), np.tile(sinA, (2, 1)).copy()


def _head_perm():
    """Even/odd interleave of RoPE partner dims, per head (384 cols)."""
    perm = np.empty(384, np.int64)
    for h in range(HPC):
        for j in range(32):
            perm[64 * h + 2 * j] = 64 * h + j
            perm[64 * h + 2 * j + 1] = 64 * h + j + 32
    return perm


def make_in_maps(x, Wqkv, Wproj):
    x = np.asarray(x, np.float32)
    Wqkv = np.asarray(Wqkv, np.float32)
    Wproj = np.asarray(Wproj, np.float32)
    wq, wk, wv = Wqkv[:, 0:C], Wqkv[:, C : 2 * C], Wqkv[:, 2 * C : 3 * C]
    cosT, sinA = _rope_tables()
    perm = _head_perm()
    m01 = (np.arange(128)[None, :] >= np.arange(128)[:, None]).astype(np.float16)
    mask01 = np.concatenate([m01, m01], axis=1)
    in_maps = []
    for core in range(8):
        b, hh = core // 2, core % 2
        cols = slice(384 * hh, 384 * (hh + 1))
        wq_c = wq[:, cols][:, perm]
        wk_c = wk[:, cols][:, perm]
        in_maps.append(
            {
                "xT": np.ascontiguousarray(x[b].T.astype(np.float16)),
                "wqk": np.ascontiguousarray(np.concatenate([wq_c, wk_c], axis=1).astype(np.float16)),
                "wv": np.ascontiguousarray(wv[:, cols].astype(np.float16)),
                "wp": np.ascontiguousarray(
                    Wproj[384 * hh : 384 * (hh + 1), :].astype(np.float16)
                ),
                "cosT": cosT.astype(np.float16),
                "sinA": sinA.astype(np.float16),
                "mask01": mask01,
                "eye": np.eye(128, dtype=np.float32),
            }
        )
    return in_maps


_NC_CACHE = None


def _get_program():
    global _NC_CACHE
    if _NC_CACHE is None:
        _NC_CACHE = build_program()
    return _NC_CACHE


def kernel(x, Wqkv, Wproj):
    nc = _get_program()
    in_maps = make_in_maps(x, Wqkv, Wproj)
    res = bass_utils.run_bass_kernel_spmd(nc, in_maps, core_ids=list(range(8)))
    out = np.empty((B, S, C), np.float32)
    for b in range(B):
        out[b] = res.results[2 * b]["out"].astype(np.float32) + res.results[
            2 * b + 1
        ]["out"].astype(np.float32)
    return out
